# revision 11
# baseline (speedup 1.0000x reference)
"""Trainium2 Bass kernel for capsule-network dynamic routing.

Problem: u [64, 2048, 16], W [2048, 16, 1024] ->
  uhat = einsum('bni,nij->bnj', u, W)  (viewed [B, N, 32, 32])
  3 routing iterations (softmax over out-caps, squash) -> v [64, 32, 32]

Sharding: n (input capsules) split across 8 cores, 256 per core.
W slice stays SBUF-resident (bf16); uhat is recomputed on the PE each
routing pass (never materialized to HBM).  The per-iteration s-reduction
([64, 1024] partial sums) is AllReduced across cores.

Layout: j is stored k-major (j' = k*32 + o, "(k,o)") so the c-weighting
(t2 = uh * c) broadcasts c over k with a packed last dim (DVE 2x mode).

Per-core n indexing: n = q*8 + 2r + h (q: 32 W blocks, r: 4 PE row
groups, h: psU partition half).  One chunk = (q, r): a single matmul
[K=32 zero-block-diag, M=128, N=1024] produces psU[64h+b, (k,o)] for
both h at once (tile_position=(32r, 0)).

Pipeline per chunk (engines overlap across chunks):
  PE:    psU [128, 1024] = uZP-block^T @ WB-block        (427 ns)
  ACT:   uh = psU -> bf16                                 (1038 ns)
  DVE:   tmp = uh * v_bf          (bf16 2x, 594 ns)
  Pool:  th = tmp[:, :512] + tmp[:, 512:]  (k 32->16 fold)
  DVE:   aa = reduce_k(th)        (f32 out, 594 ns)
  Pool:  blog_slice += aa
  per q (4 chunks): softmax on DVE/ACT -> cc (bf16)
  DVE:   t2 = uh * cc_bcast_over_k (bf16 2x, 594 ns)
  PE:    psS += I2B^T @ t2        (s accumulation, 427 ns)

Host-side layouts per core (W/u cast to bf16):
  WB  [32, 128, 1024]: WB[q, 16*p8+i, k*32+o] = W[q*8+p8, i, o*32+k]
  uB  [128, 2048]:     uB[16*p8+i, q*64+b] = u[b, q*8+p8, i]  (pass A)
  uZP [128, 4096]:     uZP[32r+16h+i, q*128+64h+b] = u[b, q*8+2r+h, i]
  I2B [128, 64]:       stacked 64x64 identities, bf16 (h/b merge)
"""

import numpy as np

B = 64
N_FULL = 2048
D_IN = 16
N_OUT = 32
D_OUT = 32
J = N_OUT * D_OUT  # 1024
N_CORES = 8
NL = N_FULL // N_CORES  # 256 local capsules
QB = NL // 8  # 32 q-blocks

_CACHE = {}


def _pack_inputs(u, W):
    """Shard along n and build per-core SBUF-friendly layouts (bf16)."""
    import ml_dtypes
    bf = ml_dtypes.bfloat16
    I2B = np.tile(np.eye(B, dtype=np.float32), (2, 1)).astype(bf)
    in_maps = []
    for c in range(N_CORES):
        ul = u[:, c * NL:(c + 1) * NL, :]          # [64, 256, 16]
        Wl = W[c * NL:(c + 1) * NL]                # [256, 16, 1024]
        # (k,o) layout: j' = k*32 + o
        Wko = np.ascontiguousarray(
            Wl.reshape(NL, D_IN, N_OUT, D_OUT).transpose(0, 1, 3, 2)
            .reshape(NL, D_IN, J))
        WB = np.ascontiguousarray(
            Wko.reshape(QB, 8, D_IN, J).reshape(QB, 128, J)).astype(bf)
        uB = np.ascontiguousarray(
            ul.reshape(B, QB, 8, D_IN).transpose(2, 3, 1, 0)
            .reshape(128, QB * B)).astype(bf)
        # uZP[32r+16h+i, q*128+64h'+b] = u[b, q*8+2r+h, i] iff h==h'
        un = ul.reshape(B, QB, 4, 2, D_IN)  # [b, q, r, h, i]
        Z = np.zeros((4, 2, D_IN, QB, 2, B), dtype=np.float32)
        for h in range(2):
            Z[:, h, :, :, h, :] = un[:, :, :, h, :].transpose(2, 3, 1, 0)
        uZP = Z.reshape(128, QB * 2 * B).astype(bf)
        in_maps.append({"WB": WB, "uB": uB, "uZP": uZP, "I2B": I2B})
    return in_maps


def _build_program():
    import concourse.bass as bass
    import concourse.tile as tile
    from concourse import bacc, mybir

    f32 = mybir.dt.float32
    bf16 = mybir.dt.bfloat16
    AF = mybir.ActivationFunctionType
    ALU = mybir.AluOpType
    AX = mybir.AxisListType

    nc = bacc.Bacc("TRN2", target_bir_lowering=False, debug=False,
                   num_devices=N_CORES)
    WB_d = nc.dram_tensor("WB", [QB, 128, J], bf16, kind="ExternalInput").ap()
    uB_d = nc.dram_tensor("uB", [128, QB * B], bf16, kind="ExternalInput").ap()
    uZP_d = nc.dram_tensor("uZP", [128, QB * 2 * B], bf16,
                           kind="ExternalInput").ap()
    I2B_d = nc.dram_tensor("I2B", [128, B], bf16, kind="ExternalInput").ap()
    v_d = nc.dram_tensor("v_out", [B, J], f32, kind="ExternalOutput").ap()

    with tile.TileContext(nc) as tc:
        with (
            tc.tile_pool(name="wpool", bufs=1) as wpool,
            tc.tile_pool(name="state", bufs=1) as state,
            tc.tile_pool(name="uhp", bufs=2) as uhp,
            tc.tile_pool(name="tmpp", bufs=3) as tmpp,
            tc.tile_pool(name="scratch", bufs=2) as scratch,
            tc.tile_pool(name="smalls", bufs=3) as smalls,
            tc.tile_pool(name="pU", bufs=3, space="PSUM") as pU,
            tc.tile_pool(name="pS", bufs=1, space="PSUM") as pS,
            tc.tile_pool(name="dram", bufs=2, space="DRAM") as dram,
        ):
            # --- load inputs ---
            w_tiles = []
            for q in range(QB):
                wt = wpool.tile([128, J], bf16, tag=f"w{q}")
                nc.sync.dma_start(wt[:], WB_d[q])
                w_tiles.append(wt)
            uB_t = state.tile([128, QB * B], bf16, tag="uB")
            nc.sync.dma_start(uB_t[:], uB_d[:])
            uZP_t = state.tile([128, QB * 2 * B], bf16, tag="uZP")
            nc.sync.dma_start(uZP_t[:], uZP_d[:])
            I2B_t = state.tile([128, B], bf16, tag="I2B")
            nc.sync.dma_start(I2B_t[:], I2B_d[:])

            # logits blog[64h+b, (q*4+r)*32+o] for n = q*8+2r+h
            blog = state.tile([128, NL // 2 * N_OUT], f32, tag="blog")
            nc.gpsimd.memset(blog[:], 0.0)
            v_t = state.tile([B, J], f32, tag="v")
            v_bf = state.tile([128, J], bf16, tag="v_bf")

            def ar_squash(merged_ps, scale):
                """merged [64,J] psum -> AllReduce -> squash -> v_t, v_bf."""
                s_loc = scratch.tile([B, J], bf16, tag="st", bufs=1)
                nc.scalar.mul(s_loc[:], merged_ps[:], scale)
                bin_ = dram.tile([B, J], bf16, tag="bounce_in")
                bout = dram.tile([B, J], bf16, tag="bounce_out")
                nc.sync.dma_start(bin_[:], s_loc[:])
                nc.gpsimd.collective_compute(
                    "AllReduce", ALU.add,
                    replica_groups=[list(range(N_CORES))],
                    ins=[bin_.opt()], outs=[bout.opt()],
                )
                s_g = scratch.tile([B, J], bf16, tag="st2", bufs=1)
                nc.sync.dma_start(s_g[:], bout[:])
                # squash: v = s * sqrt(n2)/(1+n2);  (k,o): norm over k
                sq = scratch.tile([B, J], f32, tag="st3", bufs=1)
                nc.vector.tensor_mul(sq[:], s_g[:], s_g[:])
                n2 = smalls.tile([B, N_OUT], f32, tag="n2")
                nc.vector.reduce_sum(
                    n2[:], sq[:].rearrange("p (k o) -> p o k", o=N_OUT),
                    axis=AX.X)
                n2p1 = smalls.tile([B, N_OUT], f32, tag="n2p1")
                nc.scalar.add(n2p1[:], n2[:], 1.0)
                rcp = smalls.tile([B, N_OUT], f32, tag="rcp")
                nc.vector.reciprocal(rcp[:], n2p1[:])
                rt = smalls.tile([B, N_OUT], f32, tag="rt")
                nc.scalar.activation(rt[:], n2[:], AF.Sqrt)
                scl = smalls.tile([B, N_OUT], f32, tag="scl")
                nc.vector.tensor_mul(scl[:], rt[:], rcp[:])
                nc.vector.tensor_mul(
                    v_t[:].rearrange("p (k o) -> p k o", o=N_OUT),
                    s_g[:].rearrange("p (k o) -> p k o", o=N_OUT),
                    scl[:].unsqueeze(1).broadcast_to([B, D_OUT, N_OUT]))
                nc.vector.tensor_copy(v_bf[0:B, :], v_t[:])
                nc.vector.tensor_copy(v_bf[B:2 * B, :], v_bf[0:B, :])

            # ---- pass A: s1 = (1/32) * sum_n uhat ----
            psA = pS.tile([B, J], f32, tag="psS", bufs=1)
            for q in range(QB):
                for jh in range(2):
                    nc.tensor.matmul(
                        psA[:, jh * 512:(jh + 1) * 512],
                        lhsT=uB_t[:, q * B:(q + 1) * B],
                        rhs=w_tiles[q][:, jh * 512:(jh + 1) * 512],
                        start=(q == 0), stop=(q == QB - 1))
            ar_squash(psA, 1.0 / N_OUT)

            # ---- passes B, C ----
            # a-reduce path runs self-contained per chunk on ONE engine
            # (cross-engine ping-pong per chunk costs ~1.5us stalls).
            # Pool-chunks: tmp/th folds on Pool, tiny reduce tail on DVE.

            for it in range(2):
                psS = pS.tile([B, J], f32, tag="psS", bufs=1)
                uh_live = {}

                def phase1(q):
                    uhq = []
                    n_pool = 2 if q % 2 == 0 else 1
                    for r in range(4):
                        psU = pU.tile([128, J], f32, tag="psU", bufs=3)
                        for jh in range(2):
                            nc.tensor.matmul(
                                psU[:, jh * 512:(jh + 1) * 512],
                                lhsT=uZP_t[32 * r:32 * r + 32,
                                           q * 2 * B:(q + 1) * 2 * B],
                                rhs=w_tiles[q][32 * r:32 * r + 32,
                                               jh * 512:(jh + 1) * 512],
                                start=True, stop=True,
                                tile_position=(32 * r, 0))
                        uh = uhp.tile([128, J], bf16, tag=f"uh{r}")
                        nc.scalar.mul(uh[:], psU[:], 1.0)
                        aa = smalls.tile([128, N_OUT], f32, tag="aa")
                        tmp = tmpp.tile([128, J], bf16, tag="tmp")
                        if r < n_pool:
                            # fully on Pool: mult + 3 halving folds, DVE tail
                            nc.gpsimd.tensor_mul(tmp[:], uh[:], v_bf[:])
                            th = tmpp.tile([128, J // 2], bf16, tag="th")
                            nc.gpsimd.tensor_add(
                                th[:], tmp[:, 0:512], tmp[:, 512:1024])
                            th2 = tmpp.tile([128, J // 4], bf16, tag="th2")
                            nc.gpsimd.tensor_add(
                                th2[:], th[:, 0:256], th[:, 256:512])
                            th3 = tmpp.tile([128, J // 8], bf16, tag="th3")
                            nc.gpsimd.tensor_add(
                                th3[:], th2[:, 0:128], th2[:, 128:256])
                            nc.vector.reduce_sum(
                                aa[:],
                                th3[:].rearrange("p (k o) -> p o k", o=N_OUT),
                                axis=AX.X)
                        else:
                            # fully on DVE: mult + 1 fold + reduce
                            nc.vector.tensor_mul(tmp[:], uh[:], v_bf[:])
                            th = tmpp.tile([128, J // 2], bf16, tag="thd")
                            nc.vector.tensor_add(
                                th[:], tmp[:, 0:512], tmp[:, 512:1024])
                            nc.vector.reduce_sum(
                                aa[:],
                                th[:].rearrange("p (k o) -> p o k", o=N_OUT),
                                axis=AX.X)
                        bsl = blog[:, (q * 4 + r) * N_OUT:
                                   (q * 4 + r + 1) * N_OUT]
                        nc.gpsimd.tensor_add(bsl, bsl, aa[:])
                        uhq.append(uh)
                    uh_live[q] = uhq

                def phase2(q, last):
                    # softmax over o for q's 8 capsules, then t2 + s-merge
                    uhq = uh_live.pop(q)
                    bq = blog[:, q * 4 * N_OUT:(q + 1) * 4 * N_OUT]
                    mx = smalls.tile([128, 4], f32, tag="mx")
                    nc.vector.reduce_max(
                        mx[:], bq.rearrange("p (r o) -> p r o", o=N_OUT),
                        axis=AX.X)
                    eein = smalls.tile([128, 4 * N_OUT], f32, tag="eein")
                    nc.vector.tensor_tensor(
                        eein[:].rearrange("p (r o) -> p r o", o=N_OUT),
                        bq.rearrange("p (r o) -> p r o", o=N_OUT),
                        mx[:].unsqueeze(2).broadcast_to([128, 4, N_OUT]),
                        op=ALU.subtract)
                    ee = smalls.tile([128, 4 * N_OUT], f32, tag="ee")
                    nc.scalar.activation(ee[:], eein[:], AF.Exp)
                    sm = smalls.tile([128, 4], f32, tag="sm")
                    nc.vector.reduce_sum(
                        sm[:], ee[:].rearrange("p (r o) -> p r o", o=N_OUT),
                        axis=AX.X)
                    rc = smalls.tile([128, 4], f32, tag="rc")
                    nc.vector.reciprocal(rc[:], sm[:])
                    cc = smalls.tile([128, 4 * N_OUT], bf16, tag="cc")
                    nc.vector.tensor_tensor(
                        cc[:].rearrange("p (r o) -> p r o", o=N_OUT),
                        ee[:].rearrange("p (r o) -> p r o", o=N_OUT),
                        rc[:].unsqueeze(2).broadcast_to([128, 4, N_OUT]),
                        op=ALU.mult)
                    for r in range(4):
                        t2 = tmpp.tile([128, J], bf16, tag="t2")
                        nc.vector.tensor_tensor(
                            t2[:].rearrange("p (k o) -> p k o", o=N_OUT),
                            uhq[r][:].rearrange("p (k o) -> p k o", o=N_OUT),
                            cc[:, r * N_OUT:(r + 1) * N_OUT]
                            .unsqueeze(1).broadcast_to([128, D_OUT, N_OUT]),
                            op=ALU.mult)
                        for jh in range(2):
                            nc.tensor.matmul(
                                psS[:, jh * 512:(jh + 1) * 512],
                                lhsT=I2B_t[:],
                                rhs=t2[:, jh * 512:(jh + 1) * 512],
                                start=(q == 0 and r == 0),
                                stop=(last and r == 3))

                for q in range(QB):
                    phase1(q)
                    if q >= 1:
                        phase2(q - 1, last=False)
                phase2(QB - 1, last=True)
                ar_squash(psS, 1.0)

            nc.sync.dma_start(v_d[:], v_t[:])

    nc.compile()
    return nc


def _get_program():
    if "nc" not in _CACHE:
        _CACHE["nc"] = _build_program()
    return _CACHE["nc"]


def kernel(u, W):
    from concourse.bass_utils import run_bass_kernel_spmd

    nc = _get_program()
    in_maps = _pack_inputs(np.asarray(u, np.float32), np.asarray(W, np.float32))
    res = run_bass_kernel_spmd(nc, in_maps, list(range(N_CORES)))
    v = res.results[0]["v_out"]
    # (k,o) layout -> [b, o, k]
    return np.ascontiguousarray(
        v.reshape(B, D_OUT, N_OUT).transpose(0, 2, 1))


# revision 16
# speedup vs baseline: 1.0949x; 1.0949x over previous
"""Trainium2 Bass kernel for capsule-network dynamic routing.

Problem: u [64, 2048, 16], W [2048, 16, 1024] ->
  uhat = einsum('bni,nij->bnj', u, W)  (viewed [B, N, 32, 32])
  3 routing iterations (softmax over out-caps, squash) -> v [64, 32, 32]

Sharding: n (input capsules) split across 8 cores, 256 per core.
W slice stays SBUF-resident (bf16); uhat is recomputed on the PE each
routing pass (never materialized to HBM).  The per-iteration s-reduction
([64, 1024] partial sums) is AllReduced across cores.

Layout: j is stored k-major (j' = k*32 + o, "(k,o)") so the c-weighting
(t2 = uh * c) broadcasts c over k with a packed last dim (DVE 2x mode).

Per-core n indexing: n = q*8 + 2r + h (q: 32 W blocks, r: 4 PE row
groups, h: psU partition half).  One chunk = (q, r): a single matmul
[K=32 zero-block-diag, M=128, N=1024] produces psU[64h+b, (k,o)] for
both h at once (tile_position=(32r, 0)).

Pipeline per chunk (engines overlap across chunks):
  PE:    psU [128, 1024] = uZP-block^T @ WB-block        (427 ns)
  ACT:   uh = psU -> bf16                                 (1038 ns)
  DVE:   tmp = uh * v_bf          (bf16 2x, 594 ns)
  Pool:  th = tmp[:, :512] + tmp[:, 512:]  (k 32->16 fold)
  DVE:   aa = reduce_k(th)        (f32 out, 594 ns)
  Pool:  blog_slice += aa
  per q (4 chunks): softmax on DVE/ACT -> cc (bf16)
  DVE:   t2 = uh * cc_bcast_over_k (bf16 2x, 594 ns)
  PE:    psS += I2B^T @ t2        (s accumulation, 427 ns)

Host-side layouts per core (W/u cast to bf16):
  WB  [32, 128, 1024]: WB[q, 16*p8+i, k*32+o] = W[q*8+p8, i, o*32+k]
  uB  [128, 2048]:     uB[16*p8+i, q*64+b] = u[b, q*8+p8, i]  (pass A)
  uZP [128, 4096]:     uZP[32r+16h+i, q*128+64h+b] = u[b, q*8+2r+h, i]
  I2B [128, 64]:       stacked 64x64 identities, bf16 (h/b merge)
"""

import numpy as np

B = 64
N_FULL = 2048
D_IN = 16
N_OUT = 32
D_OUT = 32
J = N_OUT * D_OUT  # 1024
N_CORES = 8
NL = N_FULL // N_CORES  # 256 local capsules
QB = NL // 8  # 32 q-blocks

_CACHE = {}


def _pack_inputs(u, W):
    """Shard along n and build per-core SBUF-friendly layouts (bf16)."""
    import ml_dtypes
    bf = ml_dtypes.bfloat16
    I2B = np.tile(np.eye(B, dtype=np.float32), (2, 1)).astype(bf)
    in_maps = []
    for c in range(N_CORES):
        ul = u[:, c * NL:(c + 1) * NL, :]          # [64, 256, 16]
        Wl = W[c * NL:(c + 1) * NL]                # [256, 16, 1024]
        # (k,o) layout: j' = k*32 + o
        Wko = np.ascontiguousarray(
            Wl.reshape(NL, D_IN, N_OUT, D_OUT).transpose(0, 1, 3, 2)
            .reshape(NL, D_IN, J))
        WB = np.ascontiguousarray(
            Wko.reshape(QB, 8, D_IN, J).reshape(QB, 128, J)).astype(bf)
        uB = np.ascontiguousarray(
            ul.reshape(B, QB, 8, D_IN).transpose(2, 3, 1, 0)
            .reshape(128, QB * B)).astype(bf)
        # uZP[32r+16h+i, q*128+64h'+b] = u[b, q*8+2r+h, i] iff h==h'
        un = ul.reshape(B, QB, 4, 2, D_IN)  # [b, q, r, h, i]
        Z = np.zeros((4, 2, D_IN, QB, 2, B), dtype=np.float32)
        for h in range(2):
            Z[:, h, :, :, h, :] = un[:, :, :, h, :].transpose(2, 3, 1, 0)
        uZP = Z.reshape(128, QB * 2 * B).astype(bf)
        in_maps.append({"WB": WB, "uB": uB, "uZP": uZP, "I2B": I2B})
    return in_maps


def _build_program():
    import concourse.bass as bass
    import concourse.tile as tile
    from concourse import bacc, mybir

    f32 = mybir.dt.float32
    bf16 = mybir.dt.bfloat16
    AF = mybir.ActivationFunctionType
    ALU = mybir.AluOpType
    AX = mybir.AxisListType

    nc = bacc.Bacc("TRN2", target_bir_lowering=False, debug=False,
                   num_devices=N_CORES)
    WB_d = nc.dram_tensor("WB", [QB, 128, J], bf16, kind="ExternalInput").ap()
    uB_d = nc.dram_tensor("uB", [128, QB * B], bf16, kind="ExternalInput").ap()
    uZP_d = nc.dram_tensor("uZP", [128, QB * 2 * B], bf16,
                           kind="ExternalInput").ap()
    I2B_d = nc.dram_tensor("I2B", [128, B], bf16, kind="ExternalInput").ap()
    v_d = nc.dram_tensor("v_out", [B, J], f32, kind="ExternalOutput").ap()

    with tile.TileContext(nc) as tc:
        with (
            tc.tile_pool(name="wpool", bufs=1) as wpool,
            tc.tile_pool(name="state", bufs=1) as state,
            tc.tile_pool(name="uhp", bufs=3) as uhp,
            tc.tile_pool(name="tmpp", bufs=3) as tmpp,
            tc.tile_pool(name="scratch", bufs=2) as scratch,
            tc.tile_pool(name="smalls", bufs=3) as smalls,
            tc.tile_pool(name="pU", bufs=3, space="PSUM") as pU,
            tc.tile_pool(name="pS", bufs=1, space="PSUM") as pS,
            tc.tile_pool(name="dram", bufs=2, space="DRAM") as dram,
        ):
            # --- load inputs ---
            w_tiles = []
            for q in range(QB):
                wt = wpool.tile([128, J], bf16, tag=f"w{q}")
                nc.sync.dma_start(wt[:], WB_d[q])
                w_tiles.append(wt)
            uB_t = state.tile([128, QB * B], bf16, tag="uB")
            nc.sync.dma_start(uB_t[:], uB_d[:])
            uZP_t = state.tile([128, QB * 2 * B], bf16, tag="uZP")
            nc.sync.dma_start(uZP_t[:], uZP_d[:])
            I2B_t = state.tile([128, B], bf16, tag="I2B")
            nc.sync.dma_start(I2B_t[:], I2B_d[:])

            # logits blog[64h+b, (q*4+r)*32+o] for n = q*8+2r+h
            blog = state.tile([128, NL // 2 * N_OUT], f32, tag="blog")
            nc.gpsimd.memset(blog[:], 0.0)
            v_t = state.tile([B, J], f32, tag="v")
            v_bf = state.tile([128, J], bf16, tag="v_bf")

            def ar_squash(merged_ps, scale):
                """merged [64,J] psum -> AllReduce -> squash -> v_t, v_bf."""
                s_loc = scratch.tile([B, J], bf16, tag="st", bufs=1)
                nc.scalar.mul(s_loc[:], merged_ps[:], scale)
                bin_ = dram.tile([B, J], bf16, tag="bounce_in")
                bout = dram.tile([B, J], bf16, tag="bounce_out")
                nc.sync.dma_start(bin_[:], s_loc[:])
                nc.gpsimd.collective_compute(
                    "AllReduce", ALU.add,
                    replica_groups=[list(range(N_CORES))],
                    ins=[bin_.opt()], outs=[bout.opt()],
                )
                s_g = scratch.tile([B, J], bf16, tag="st2", bufs=1)
                nc.sync.dma_start(s_g[:], bout[:])
                # squash: v = s * sqrt(n2)/(1+n2);  (k,o): norm over k
                sq = scratch.tile([B, J], f32, tag="st3", bufs=1)
                nc.vector.tensor_mul(sq[:], s_g[:], s_g[:])
                n2 = smalls.tile([B, N_OUT], f32, tag="n2")
                nc.vector.reduce_sum(
                    n2[:], sq[:].rearrange("p (k o) -> p o k", o=N_OUT),
                    axis=AX.X)
                n2p1 = smalls.tile([B, N_OUT], f32, tag="n2p1")
                nc.scalar.add(n2p1[:], n2[:], 1.0)
                rcp = smalls.tile([B, N_OUT], f32, tag="rcp")
                nc.vector.reciprocal(rcp[:], n2p1[:])
                rt = smalls.tile([B, N_OUT], f32, tag="rt")
                nc.scalar.activation(rt[:], n2[:], AF.Sqrt)
                scl = smalls.tile([B, N_OUT], f32, tag="scl")
                nc.vector.tensor_mul(scl[:], rt[:], rcp[:])
                nc.vector.tensor_mul(
                    v_t[:].rearrange("p (k o) -> p k o", o=N_OUT),
                    s_g[:].rearrange("p (k o) -> p k o", o=N_OUT),
                    scl[:].unsqueeze(1).broadcast_to([B, D_OUT, N_OUT]))
                nc.vector.tensor_copy(v_bf[0:B, :], v_t[:])
                nc.vector.tensor_copy(v_bf[B:2 * B, :], v_bf[0:B, :])

            # ---- pass A: s1 = (1/32) * sum_n uhat ----
            psA = pS.tile([B, J], f32, tag="psS", bufs=1)
            for q in range(QB):
                for jh in range(2):
                    nc.tensor.matmul(
                        psA[:, jh * 512:(jh + 1) * 512],
                        lhsT=uB_t[:, q * B:(q + 1) * B],
                        rhs=w_tiles[q][:, jh * 512:(jh + 1) * 512],
                        start=(q == 0), stop=(q == QB - 1))
            ar_squash(psA, 1.0 / N_OUT)

            # ---- passes B, C ----
            # a-reduce path runs self-contained per chunk on ONE engine
            # (cross-engine ping-pong per chunk costs ~1.5us stalls).
            # Pool-chunks: tmp/th folds on Pool, tiny reduce tail on DVE.

            for it in range(2):
                psS = pS.tile([B, J], f32, tag="psS", bufs=1)
                uh_live = {}

                def phase1(q):
                    uhq = []
                    n_pool = 2 if q % 2 == 0 else 1
                    for r in range(4):
                        psU = pU.tile([128, J], f32, tag="psU", bufs=3)
                        for jh in range(2):
                            nc.tensor.matmul(
                                psU[:, jh * 512:(jh + 1) * 512],
                                lhsT=uZP_t[32 * r:32 * r + 32,
                                           q * 2 * B:(q + 1) * 2 * B],
                                rhs=w_tiles[q][32 * r:32 * r + 32,
                                               jh * 512:(jh + 1) * 512],
                                start=True, stop=True,
                                tile_position=(32 * r, 0))
                        uh = uhp.tile([128, J], bf16, tag=f"uh{r}")
                        nc.scalar.mul(uh[:], psU[:], 1.0)
                        aa = smalls.tile([128, N_OUT], f32, tag="aa")
                        tmp = tmpp.tile([128, J], bf16, tag="tmp")
                        if r < n_pool:
                            # fully on Pool: mult + 3 halving folds, DVE tail
                            nc.gpsimd.tensor_mul(tmp[:], uh[:], v_bf[:])
                            th = tmpp.tile([128, J // 2], bf16, tag="th")
                            nc.gpsimd.tensor_add(
                                th[:], tmp[:, 0:512], tmp[:, 512:1024])
                            th2 = tmpp.tile([128, J // 4], bf16, tag="th2")
                            nc.gpsimd.tensor_add(
                                th2[:], th[:, 0:256], th[:, 256:512])
                            th3 = tmpp.tile([128, J // 8], bf16, tag="th3")
                            nc.gpsimd.tensor_add(
                                th3[:], th2[:, 0:128], th2[:, 128:256])
                            nc.vector.reduce_sum(
                                aa[:],
                                th3[:].rearrange("p (k o) -> p o k", o=N_OUT),
                                axis=AX.X)
                        else:
                            # fully on DVE: mult + 1 fold + reduce
                            nc.vector.tensor_mul(tmp[:], uh[:], v_bf[:])
                            th = tmpp.tile([128, J // 2], bf16, tag="thd")
                            nc.vector.tensor_add(
                                th[:], tmp[:, 0:512], tmp[:, 512:1024])
                            nc.vector.reduce_sum(
                                aa[:],
                                th[:].rearrange("p (k o) -> p o k", o=N_OUT),
                                axis=AX.X)
                        bsl = blog[:, (q * 4 + r) * N_OUT:
                                   (q * 4 + r + 1) * N_OUT]
                        nc.gpsimd.tensor_add(bsl, bsl, aa[:])
                        uhq.append(uh)
                    uh_live[q] = uhq

                ee_live = {}

                def phase2_head(q):
                    # softmax up to exp: issue BEFORE next q's evacs so the
                    # exp doesn't queue behind them on ACT
                    bq = blog[:, q * 4 * N_OUT:(q + 1) * 4 * N_OUT]
                    mx = smalls.tile([128, 4], f32, tag="mx")
                    nc.vector.reduce_max(
                        mx[:], bq.rearrange("p (r o) -> p r o", o=N_OUT),
                        axis=AX.X)
                    eein = smalls.tile([128, 4 * N_OUT], f32, tag="eein")
                    nc.vector.tensor_tensor(
                        eein[:].rearrange("p (r o) -> p r o", o=N_OUT),
                        bq.rearrange("p (r o) -> p r o", o=N_OUT),
                        mx[:].unsqueeze(2).broadcast_to([128, 4, N_OUT]),
                        op=ALU.subtract)
                    ee = smalls.tile([128, 4 * N_OUT], f32, tag="ee")
                    nc.scalar.activation(ee[:], eein[:], AF.Exp)
                    ee_live[q] = ee

                def phase2(q, last):
                    # softmax tail, then t2 + s-merge
                    uhq = uh_live.pop(q)
                    ee = ee_live.pop(q)
                    sm = smalls.tile([128, 4], f32, tag="sm")
                    nc.vector.reduce_sum(
                        sm[:], ee[:].rearrange("p (r o) -> p r o", o=N_OUT),
                        axis=AX.X)
                    rc = smalls.tile([128, 4], f32, tag="rc")
                    nc.vector.reciprocal(rc[:], sm[:])
                    cc = smalls.tile([128, 4 * N_OUT], bf16, tag="cc")
                    nc.vector.tensor_tensor(
                        cc[:].rearrange("p (r o) -> p r o", o=N_OUT),
                        ee[:].rearrange("p (r o) -> p r o", o=N_OUT),
                        rc[:].unsqueeze(2).broadcast_to([128, 4, N_OUT]),
                        op=ALU.mult)
                    for r in range(4):
                        t2 = tmpp.tile([128, J], bf16, tag="t2")
                        nc.vector.tensor_tensor(
                            t2[:].rearrange("p (k o) -> p k o", o=N_OUT),
                            uhq[r][:].rearrange("p (k o) -> p k o", o=N_OUT),
                            cc[:, r * N_OUT:(r + 1) * N_OUT]
                            .unsqueeze(1).broadcast_to([128, D_OUT, N_OUT]),
                            op=ALU.mult)
                        for jh in range(2):
                            nc.tensor.matmul(
                                psS[:, jh * 512:(jh + 1) * 512],
                                lhsT=I2B_t[:],
                                rhs=t2[:, jh * 512:(jh + 1) * 512],
                                start=(q == 0 and r == 0),
                                stop=(last and r == 3))

                for q in range(QB):
                    phase1(q)
                    phase2_head(q)
                    if q >= 1:
                        phase2(q - 1, last=False)
                phase2(QB - 1, last=True)
                ar_squash(psS, 1.0)

            nc.sync.dma_start(v_d[:], v_t[:])

    nc.compile()
    return nc


def _get_program():
    if "nc" not in _CACHE:
        _CACHE["nc"] = _build_program()
    return _CACHE["nc"]


def kernel(u, W):
    from concourse.bass_utils import run_bass_kernel_spmd

    nc = _get_program()
    in_maps = _pack_inputs(np.asarray(u, np.float32), np.asarray(W, np.float32))
    res = run_bass_kernel_spmd(nc, in_maps, list(range(N_CORES)))
    v = res.results[0]["v_out"]
    # (k,o) layout -> [b, o, k]
    return np.ascontiguousarray(
        v.reshape(B, D_OUT, N_OUT).transpose(0, 2, 1))


# revision 18
# speedup vs baseline: 1.1466x; 1.0472x over previous
"""Trainium2 Bass kernel for capsule-network dynamic routing.

Problem: u [64, 2048, 16], W [2048, 16, 1024] ->
  uhat = einsum('bni,nij->bnj', u, W)  (viewed [B, N, 32, 32])
  3 routing iterations (softmax over out-caps, squash) -> v [64, 32, 32]

Sharding: n (input capsules) split across 8 cores, 256 per core.
W slice stays SBUF-resident (bf16); uhat is recomputed on the PE each
routing pass (never materialized to HBM).  The per-iteration s-reduction
([64, 1024] partial sums) is AllReduced across cores.

Layout: j is stored k-major (j' = k*32 + o, "(k,o)") so the c-weighting
(t2 = uh * c) broadcasts c over k with a packed last dim (DVE 2x mode).

Per-core n indexing: n = q*8 + 2r + h (q: 32 W blocks, r: 4 PE row
groups, h: psU partition half).  One chunk = (q, r): a single matmul
[K=32 zero-block-diag, M=128, N=1024] produces psU[64h+b, (k,o)] for
both h at once (tile_position=(32r, 0)).

Pipeline per chunk (engines overlap across chunks):
  PE:    psU [128, 1024] = uZP-block^T @ WB-block        (427 ns)
  ACT:   uh = psU -> bf16                                 (1038 ns)
  DVE:   tmp = uh * v_bf          (bf16 2x, 594 ns)
  Pool:  th = tmp[:, :512] + tmp[:, 512:]  (k 32->16 fold)
  DVE:   aa = reduce_k(th)        (f32 out, 594 ns)
  Pool:  blog_slice += aa
  per q (4 chunks): softmax on DVE/ACT -> cc (bf16)
  DVE:   t2 = uh * cc_bcast_over_k (bf16 2x, 594 ns)
  PE:    psS += I2B^T @ t2        (s accumulation, 427 ns)

Host-side layouts per core (W/u cast to bf16):
  WB  [32, 128, 1024]: WB[q, 16*p8+i, k*32+o] = W[q*8+p8, i, o*32+k]
  uB  [128, 2048]:     uB[16*p8+i, q*64+b] = u[b, q*8+p8, i]  (pass A)
  uZP [128, 4096]:     uZP[32r+16h+i, q*128+64h+b] = u[b, q*8+2r+h, i]
  I2B [128, 64]:       stacked 64x64 identities, bf16 (h/b merge)
"""

import numpy as np

B = 64
N_FULL = 2048
D_IN = 16
N_OUT = 32
D_OUT = 32
J = N_OUT * D_OUT  # 1024
N_CORES = 8
NL = N_FULL // N_CORES  # 256 local capsules
QB = NL // 8  # 32 q-blocks

_CACHE = {}


def _pack_inputs(u, W):
    """Shard along n and build per-core SBUF-friendly layouts (bf16)."""
    import ml_dtypes
    bf = ml_dtypes.bfloat16
    I2B = np.tile(np.eye(B, dtype=np.float32), (2, 1)).astype(bf)
    in_maps = []
    for c in range(N_CORES):
        ul = u[:, c * NL:(c + 1) * NL, :]          # [64, 256, 16]
        Wl = W[c * NL:(c + 1) * NL]                # [256, 16, 1024]
        # (k,o) layout: j' = k*32 + o
        Wko = np.ascontiguousarray(
            Wl.reshape(NL, D_IN, N_OUT, D_OUT).transpose(0, 1, 3, 2)
            .reshape(NL, D_IN, J))
        WB = np.ascontiguousarray(
            Wko.reshape(QB, 8, D_IN, J).reshape(QB, 128, J)).astype(bf)
        uB = np.ascontiguousarray(
            ul.reshape(B, QB, 8, D_IN).transpose(2, 3, 1, 0)
            .reshape(128, QB * B)).astype(bf)
        # uZP[32r+16h+i, q*128+64h'+b] = u[b, q*8+2r+h, i] iff h==h'
        un = ul.reshape(B, QB, 4, 2, D_IN)  # [b, q, r, h, i]
        Z = np.zeros((4, 2, D_IN, QB, 2, B), dtype=np.float32)
        for h in range(2):
            Z[:, h, :, :, h, :] = un[:, :, :, h, :].transpose(2, 3, 1, 0)
        uZP = Z.reshape(128, QB * 2 * B).astype(bf)
        in_maps.append({"WB": WB, "uB": uB, "uZP": uZP, "I2B": I2B})
    return in_maps


def _build_program():
    import concourse.bass as bass
    import concourse.tile as tile
    from concourse import bacc, mybir

    f32 = mybir.dt.float32
    bf16 = mybir.dt.bfloat16
    AF = mybir.ActivationFunctionType
    ALU = mybir.AluOpType
    AX = mybir.AxisListType

    nc = bacc.Bacc("TRN2", target_bir_lowering=False, debug=False,
                   num_devices=N_CORES)
    WB_d = nc.dram_tensor("WB", [QB, 128, J], bf16, kind="ExternalInput").ap()
    uB_d = nc.dram_tensor("uB", [128, QB * B], bf16, kind="ExternalInput").ap()
    uZP_d = nc.dram_tensor("uZP", [128, QB * 2 * B], bf16,
                           kind="ExternalInput").ap()
    I2B_d = nc.dram_tensor("I2B", [128, B], bf16, kind="ExternalInput").ap()
    v_d = nc.dram_tensor("v_out", [B, J], f32, kind="ExternalOutput").ap()

    with tile.TileContext(nc) as tc:
        with (
            tc.tile_pool(name="wpool", bufs=1) as wpool,
            tc.tile_pool(name="state", bufs=1) as state,
            tc.tile_pool(name="uhp", bufs=3) as uhp,
            tc.tile_pool(name="tmpp", bufs=3) as tmpp,
            tc.tile_pool(name="scratch", bufs=2) as scratch,
            tc.tile_pool(name="smalls", bufs=3) as smalls,
            tc.tile_pool(name="pU", bufs=3, space="PSUM") as pU,
            tc.tile_pool(name="pS", bufs=1, space="PSUM") as pS,
            tc.tile_pool(name="dram", bufs=2, space="DRAM") as dram,
        ):
            # --- load inputs ---
            uB_t = state.tile([128, QB * B], bf16, tag="uB")
            nc.sync.dma_start(uB_t[:], uB_d[:])
            uZP_t = state.tile([128, QB * 2 * B], bf16, tag="uZP")
            nc.sync.dma_start(uZP_t[:], uZP_d[:])
            I2B_t = state.tile([128, B], bf16, tag="I2B")
            nc.sync.dma_start(I2B_t[:], I2B_d[:])
            w_tiles = []
            for q in range(QB):
                wt = wpool.tile([128, J], bf16, tag=f"w{q}")
                nc.sync.dma_start(wt[:], WB_d[q])
                w_tiles.append(wt)

            # logits blog[64h+b, (q*4+r)*32+o] for n = q*8+2r+h
            blog = state.tile([128, NL // 2 * N_OUT], f32, tag="blog")
            nc.gpsimd.memset(blog[:], 0.0)
            v_t = state.tile([B, J], f32, tag="v")
            v_bf = state.tile([128, J], bf16, tag="v_bf")

            def ar_squash(merged_ps, scale):
                """merged [64,J] psum -> AllReduce -> squash -> v_t, v_bf."""
                s_loc = scratch.tile([B, J], bf16, tag="st", bufs=1)
                nc.scalar.mul(s_loc[:], merged_ps[:], scale)
                bin_ = dram.tile([B, J], bf16, tag="bounce_in")
                bout = dram.tile([B, J], bf16, tag="bounce_out")
                nc.sync.dma_start(bin_[:], s_loc[:])
                nc.gpsimd.collective_compute(
                    "AllReduce", ALU.add,
                    replica_groups=[list(range(N_CORES))],
                    ins=[bin_.opt()], outs=[bout.opt()],
                )
                s_g = scratch.tile([B, J], bf16, tag="st2", bufs=1)
                nc.sync.dma_start(s_g[:], bout[:])
                # squash: v = s * sqrt(n2)/(1+n2);  (k,o): norm over k
                sq = scratch.tile([B, J], f32, tag="st3", bufs=1)
                nc.vector.tensor_mul(sq[:], s_g[:], s_g[:])
                n2 = smalls.tile([B, N_OUT], f32, tag="n2")
                nc.vector.reduce_sum(
                    n2[:], sq[:].rearrange("p (k o) -> p o k", o=N_OUT),
                    axis=AX.X)
                n2p1 = smalls.tile([B, N_OUT], f32, tag="n2p1")
                nc.scalar.add(n2p1[:], n2[:], 1.0)
                rcp = smalls.tile([B, N_OUT], f32, tag="rcp")
                nc.vector.reciprocal(rcp[:], n2p1[:])
                rt = smalls.tile([B, N_OUT], f32, tag="rt")
                nc.scalar.activation(rt[:], n2[:], AF.Sqrt)
                scl = smalls.tile([B, N_OUT], f32, tag="scl")
                nc.vector.tensor_mul(scl[:], rt[:], rcp[:])
                nc.vector.tensor_mul(
                    v_t[:].rearrange("p (k o) -> p k o", o=N_OUT),
                    s_g[:].rearrange("p (k o) -> p k o", o=N_OUT),
                    scl[:].unsqueeze(1).broadcast_to([B, D_OUT, N_OUT]))
                nc.vector.tensor_copy(v_bf[0:B, :], v_t[:])
                nc.vector.tensor_copy(v_bf[B:2 * B, :], v_bf[0:B, :])

            # ---- pass A: s1 = (1/32) * sum_n uhat ----
            psA = pS.tile([B, J], f32, tag="psS", bufs=1)
            for q in range(QB):
                for jh in range(2):
                    nc.tensor.matmul(
                        psA[:, jh * 512:(jh + 1) * 512],
                        lhsT=uB_t[:, q * B:(q + 1) * B],
                        rhs=w_tiles[q][:, jh * 512:(jh + 1) * 512],
                        start=(q == 0), stop=(q == QB - 1))
            ar_squash(psA, 1.0 / N_OUT)

            # ---- passes B, C ----
            # a-reduce path runs self-contained per chunk on ONE engine
            # (cross-engine ping-pong per chunk costs ~1.5us stalls).
            # Pool-chunks: tmp/th folds on Pool, tiny reduce tail on DVE.

            for it in range(2):
                psS = pS.tile([B, J], f32, tag="psS", bufs=1)
                uh_live = {}

                def phase1(q):
                    uhq = [None] * 4
                    n_pool = 2 if q % 2 == 0 else 1
                    # aq collects the q's 4 aa slices; one batched blog add
                    aq = smalls.tile([128, 4 * N_OUT], f32, tag="aq")
                    pool_tails = []

                    def chunk_front(r):
                        psU = pU.tile([128, J], f32, tag="psU", bufs=3)
                        for jh in range(2):
                            nc.tensor.matmul(
                                psU[:, jh * 512:(jh + 1) * 512],
                                lhsT=uZP_t[32 * r:32 * r + 32,
                                           q * 2 * B:(q + 1) * 2 * B],
                                rhs=w_tiles[q][32 * r:32 * r + 32,
                                               jh * 512:(jh + 1) * 512],
                                start=True, stop=True,
                                tile_position=(32 * r, 0))
                        uh = uhp.tile([128, J], bf16, tag=f"uh{r}")
                        nc.scalar.mul(uh[:], psU[:], 1.0)
                        uhq[r] = uh
                        return uh

                    # Pool-chunks first: start Pool's long chains early
                    for r in range(n_pool):
                        uh = chunk_front(r)
                        tmp = tmpp.tile([128, J], bf16, tag="tmp")
                        nc.gpsimd.tensor_mul(tmp[:], uh[:], v_bf[:])
                        th = tmpp.tile([128, J // 2], bf16, tag="th")
                        nc.gpsimd.tensor_add(
                            th[:], tmp[:, 0:512], tmp[:, 512:1024])
                        th2 = tmpp.tile([128, J // 4], bf16, tag="th2")
                        nc.gpsimd.tensor_add(
                            th2[:], th[:, 0:256], th[:, 256:512])
                        th3 = tmpp.tile([128, J // 8], bf16, tag="th3")
                        nc.gpsimd.tensor_add(
                            th3[:], th2[:, 0:128], th2[:, 128:256])
                        pool_tails.append((r, th3))
                    # DVE-chunks: self-contained on DVE
                    for r in range(n_pool, 4):
                        uh = chunk_front(r)
                        tmp = tmpp.tile([128, J], bf16, tag="tmp")
                        nc.vector.tensor_mul(tmp[:], uh[:], v_bf[:])
                        th = tmpp.tile([128, J // 2], bf16, tag="thd")
                        nc.vector.tensor_add(
                            th[:], tmp[:, 0:512], tmp[:, 512:1024])
                        nc.vector.reduce_sum(
                            aq[:, r * N_OUT:(r + 1) * N_OUT],
                            th[:].rearrange("p (k o) -> p o k", o=N_OUT),
                            axis=AX.X)
                    # Pool-chunk DVE tails last (Pool chains done by now)
                    for r, th3 in pool_tails:
                        nc.vector.reduce_sum(
                            aq[:, r * N_OUT:(r + 1) * N_OUT],
                            th3[:].rearrange("p (k o) -> p o k", o=N_OUT),
                            axis=AX.X)
                    # single batched logits update for the whole q
                    bq = blog[:, q * 4 * N_OUT:(q + 1) * 4 * N_OUT]
                    nc.gpsimd.tensor_add(bq, bq, aq[:])
                    uh_live[q] = uhq

                ee_live = {}

                def phase2_head(q):
                    # softmax up to exp: issue BEFORE next q's evacs so the
                    # exp doesn't queue behind them on ACT
                    bq = blog[:, q * 4 * N_OUT:(q + 1) * 4 * N_OUT]
                    mx = smalls.tile([128, 4], f32, tag="mx")
                    nc.vector.reduce_max(
                        mx[:], bq.rearrange("p (r o) -> p r o", o=N_OUT),
                        axis=AX.X)
                    eein = smalls.tile([128, 4 * N_OUT], f32, tag="eein")
                    nc.vector.tensor_tensor(
                        eein[:].rearrange("p (r o) -> p r o", o=N_OUT),
                        bq.rearrange("p (r o) -> p r o", o=N_OUT),
                        mx[:].unsqueeze(2).broadcast_to([128, 4, N_OUT]),
                        op=ALU.subtract)
                    ee = smalls.tile([128, 4 * N_OUT], f32, tag="ee")
                    nc.scalar.activation(ee[:], eein[:], AF.Exp)
                    ee_live[q] = ee

                def phase2(q, last):
                    # softmax tail, then t2 + s-merge
                    uhq = uh_live.pop(q)
                    ee = ee_live.pop(q)
                    sm = smalls.tile([128, 4], f32, tag="sm")
                    nc.vector.reduce_sum(
                        sm[:], ee[:].rearrange("p (r o) -> p r o", o=N_OUT),
                        axis=AX.X)
                    rc = smalls.tile([128, 4], f32, tag="rc")
                    nc.vector.reciprocal(rc[:], sm[:])
                    cc = smalls.tile([128, 4 * N_OUT], bf16, tag="cc")
                    nc.vector.tensor_tensor(
                        cc[:].rearrange("p (r o) -> p r o", o=N_OUT),
                        ee[:].rearrange("p (r o) -> p r o", o=N_OUT),
                        rc[:].unsqueeze(2).broadcast_to([128, 4, N_OUT]),
                        op=ALU.mult)
                    for r in range(4):
                        t2 = tmpp.tile([128, J], bf16, tag="t2")
                        nc.vector.tensor_tensor(
                            t2[:].rearrange("p (k o) -> p k o", o=N_OUT),
                            uhq[r][:].rearrange("p (k o) -> p k o", o=N_OUT),
                            cc[:, r * N_OUT:(r + 1) * N_OUT]
                            .unsqueeze(1).broadcast_to([128, D_OUT, N_OUT]),
                            op=ALU.mult)
                        for jh in range(2):
                            nc.tensor.matmul(
                                psS[:, jh * 512:(jh + 1) * 512],
                                lhsT=I2B_t[:],
                                rhs=t2[:, jh * 512:(jh + 1) * 512],
                                start=(q == 0 and r == 0),
                                stop=(last and r == 3))

                for q in range(QB):
                    phase1(q)
                    phase2_head(q)
                    if q >= 1:
                        phase2(q - 1, last=False)
                phase2(QB - 1, last=True)
                ar_squash(psS, 1.0)

            nc.sync.dma_start(v_d[:], v_t[:])

    nc.compile()
    return nc


def _get_program():
    if "nc" not in _CACHE:
        _CACHE["nc"] = _build_program()
    return _CACHE["nc"]


def kernel(u, W):
    from concourse.bass_utils import run_bass_kernel_spmd

    nc = _get_program()
    in_maps = _pack_inputs(np.asarray(u, np.float32), np.asarray(W, np.float32))
    res = run_bass_kernel_spmd(nc, in_maps, list(range(N_CORES)))
    v = res.results[0]["v_out"]
    # (k,o) layout -> [b, o, k]
    return np.ascontiguousarray(
        v.reshape(B, D_OUT, N_OUT).transpose(0, 2, 1))


# revision 20
# speedup vs baseline: 1.2241x; 1.0676x over previous
"""Trainium2 Bass kernel for capsule-network dynamic routing.

Problem: u [64, 2048, 16], W [2048, 16, 1024] ->
  uhat = einsum('bni,nij->bnj', u, W)  (viewed [B, N, 32, 32])
  3 routing iterations (softmax over out-caps, squash) -> v [64, 32, 32]

Sharding: n (input capsules) split across 8 cores, 256 per core.
W slice stays SBUF-resident (bf16); uhat is recomputed on the PE each
routing pass (never materialized to HBM).  The per-iteration s-reduction
([64, 1024] partial sums) is AllReduced across cores.

Layout: j is stored k-major (j' = k*32 + o, "(k,o)") so the c-weighting
(t2 = uh * c) broadcasts c over k with a packed last dim (DVE 2x mode).

Per-core n indexing: n = q*8 + 2r + h (q: 32 W blocks, r: 4 PE row
groups, h: psU partition half).  One chunk = (q, r): a single matmul
[K=32 zero-block-diag, M=128, N=1024] produces psU[64h+b, (k,o)] for
both h at once (tile_position=(32r, 0)).

Pipeline per chunk (engines overlap across chunks):
  PE:    psU [128, 1024] = uZP-block^T @ WB-block        (427 ns)
  ACT:   uh = psU -> bf16                                 (1038 ns)
  DVE:   tmp = uh * v_bf          (bf16 2x, 594 ns)
  Pool:  th = tmp[:, :512] + tmp[:, 512:]  (k 32->16 fold)
  DVE:   aa = reduce_k(th)        (f32 out, 594 ns)
  Pool:  blog_slice += aa
  per q (4 chunks): softmax on DVE/ACT -> cc (bf16)
  DVE:   t2 = uh * cc_bcast_over_k (bf16 2x, 594 ns)
  PE:    psS += I2B^T @ t2        (s accumulation, 427 ns)

Host-side layouts per core (W/u cast to bf16):
  WB  [32, 128, 1024]: WB[q, 16*p8+i, k*32+o] = W[q*8+p8, i, o*32+k]
  uB  [128, 2048]:     uB[16*p8+i, q*64+b] = u[b, q*8+p8, i]  (pass A)
  uZP [128, 4096]:     uZP[32r+16h+i, q*128+64h+b] = u[b, q*8+2r+h, i]
  I2B [128, 64]:       stacked 64x64 identities, bf16 (h/b merge)
"""

import numpy as np

B = 64
N_FULL = 2048
D_IN = 16
N_OUT = 32
D_OUT = 32
J = N_OUT * D_OUT  # 1024
N_CORES = 8
NL = N_FULL // N_CORES  # 256 local capsules
QB = NL // 8  # 32 q-blocks

_CACHE = {}


def _pack_inputs(u, W):
    """Shard along n and build per-core SBUF-friendly layouts (bf16)."""
    import ml_dtypes
    bf = ml_dtypes.bfloat16
    I2B = np.tile(np.eye(B, dtype=np.float32), (2, 1)).astype(bf)
    in_maps = []
    for c in range(N_CORES):
        ul = u[:, c * NL:(c + 1) * NL, :]          # [64, 256, 16]
        Wl = W[c * NL:(c + 1) * NL]                # [256, 16, 1024]
        # (k,o) layout: j' = k*32 + o
        Wko = np.ascontiguousarray(
            Wl.reshape(NL, D_IN, N_OUT, D_OUT).transpose(0, 1, 3, 2)
            .reshape(NL, D_IN, J))
        WB = np.ascontiguousarray(
            Wko.reshape(QB, 8, D_IN, J).reshape(QB, 128, J)).astype(bf)
        uB = np.ascontiguousarray(
            ul.reshape(B, QB, 8, D_IN).transpose(2, 3, 1, 0)
            .reshape(128, QB * B)).astype(bf)
        # uZP[32r+16h+i, q*128+64h'+b] = u[b, q*8+2r+h, i] iff h==h'
        un = ul.reshape(B, QB, 4, 2, D_IN)  # [b, q, r, h, i]
        Z = np.zeros((4, 2, D_IN, QB, 2, B), dtype=np.float32)
        for h in range(2):
            Z[:, h, :, :, h, :] = un[:, :, :, h, :].transpose(2, 3, 1, 0)
        uZP = Z.reshape(128, QB * 2 * B).astype(bf)
        in_maps.append({"WB": WB, "uB": uB, "uZP": uZP, "I2B": I2B})
    return in_maps


def _build_program():
    import concourse.bass as bass
    import concourse.tile as tile
    from concourse import bacc, mybir

    f32 = mybir.dt.float32
    bf16 = mybir.dt.bfloat16
    AF = mybir.ActivationFunctionType
    ALU = mybir.AluOpType
    AX = mybir.AxisListType

    nc = bacc.Bacc("TRN2", target_bir_lowering=False, debug=False,
                   num_devices=N_CORES)
    WB_d = nc.dram_tensor("WB", [QB, 128, J], bf16, kind="ExternalInput").ap()
    uB_d = nc.dram_tensor("uB", [128, QB * B], bf16, kind="ExternalInput").ap()
    uZP_d = nc.dram_tensor("uZP", [128, QB * 2 * B], bf16,
                           kind="ExternalInput").ap()
    I2B_d = nc.dram_tensor("I2B", [128, B], bf16, kind="ExternalInput").ap()
    v_d = nc.dram_tensor("v_out", [B, J], f32, kind="ExternalOutput").ap()

    with tile.TileContext(nc) as tc:
        with (
            tc.tile_pool(name="wpool", bufs=1) as wpool,
            tc.tile_pool(name="state", bufs=1) as state,
            tc.tile_pool(name="uhp", bufs=3) as uhp,
            tc.tile_pool(name="tmpp", bufs=3) as tmpp,
            tc.tile_pool(name="scratch", bufs=2) as scratch,
            tc.tile_pool(name="smalls", bufs=3) as smalls,
            tc.tile_pool(name="pU", bufs=3, space="PSUM") as pU,
            tc.tile_pool(name="pS", bufs=1, space="PSUM") as pS,
            tc.tile_pool(name="dram", bufs=2, space="DRAM") as dram,
        ):
            # --- load inputs ---
            uB_t = state.tile([128, QB * B], bf16, tag="uB")
            nc.sync.dma_start(uB_t[:], uB_d[:])
            uZP_t = state.tile([128, QB * 2 * B], bf16, tag="uZP")
            nc.sync.dma_start(uZP_t[:], uZP_d[:])
            I2B_t = state.tile([128, B], bf16, tag="I2B")
            nc.sync.dma_start(I2B_t[:], I2B_d[:])
            w_tiles = []
            for q in range(QB):
                wt = wpool.tile([128, J], bf16, tag=f"w{q}")
                nc.sync.dma_start(wt[:], WB_d[q])
                w_tiles.append(wt)

            # logits blog[64h+b, (q*4+r)*32+o] for n = q*8+2r+h
            blog = state.tile([128, NL // 2 * N_OUT], f32, tag="blog")
            nc.gpsimd.memset(blog[:], 0.0)
            v_t = state.tile([B, J], f32, tag="v")
            v_bf = state.tile([128, J], bf16, tag="v_bf")

            def ar_squash(merged_ps, scale):
                """merged [64,J] psum -> AllReduce -> squash -> v_t, v_bf."""
                s_loc = scratch.tile([B, J], bf16, tag="st", bufs=1)
                nc.scalar.mul(s_loc[:], merged_ps[:], scale)
                bin_ = dram.tile([B, J], bf16, tag="bounce_in")
                bout = dram.tile([B, J], bf16, tag="bounce_out")
                nc.sync.dma_start(bin_[:], s_loc[:])
                nc.gpsimd.collective_compute(
                    "AllReduce", ALU.add,
                    replica_groups=[list(range(N_CORES))],
                    ins=[bin_.opt()], outs=[bout.opt()],
                )
                s_g = scratch.tile([B, J], bf16, tag="st2", bufs=1)
                nc.sync.dma_start(s_g[:], bout[:])
                # squash: v = s * sqrt(n2)/(1+n2);  (k,o): norm over k
                sq = scratch.tile([B, J], f32, tag="st3", bufs=1)
                nc.vector.tensor_mul(sq[:], s_g[:], s_g[:])
                n2 = smalls.tile([B, N_OUT], f32, tag="n2")
                nc.vector.reduce_sum(
                    n2[:], sq[:].rearrange("p (k o) -> p o k", o=N_OUT),
                    axis=AX.X)
                n2p1 = smalls.tile([B, N_OUT], f32, tag="n2p1")
                nc.scalar.add(n2p1[:], n2[:], 1.0)
                rcp = smalls.tile([B, N_OUT], f32, tag="rcp")
                nc.vector.reciprocal(rcp[:], n2p1[:])
                rt = smalls.tile([B, N_OUT], f32, tag="rt")
                nc.scalar.activation(rt[:], n2[:], AF.Sqrt)
                scl = smalls.tile([B, N_OUT], f32, tag="scl")
                nc.vector.tensor_mul(scl[:], rt[:], rcp[:])
                nc.vector.tensor_mul(
                    v_t[:].rearrange("p (k o) -> p k o", o=N_OUT),
                    s_g[:].rearrange("p (k o) -> p k o", o=N_OUT),
                    scl[:].unsqueeze(1).broadcast_to([B, D_OUT, N_OUT]))
                nc.vector.tensor_copy(v_bf[0:B, :], v_t[:])
                nc.vector.tensor_copy(v_bf[B:2 * B, :], v_bf[0:B, :])

            # ---- pass A: s1 = (1/32) * sum_n uhat ----
            psA = pS.tile([B, J], f32, tag="psS", bufs=1)
            for q in range(QB):
                for jh in range(2):
                    nc.tensor.matmul(
                        psA[:, jh * 512:(jh + 1) * 512],
                        lhsT=uB_t[:, q * B:(q + 1) * B],
                        rhs=w_tiles[q][:, jh * 512:(jh + 1) * 512],
                        start=(q == 0), stop=(q == QB - 1))
            ar_squash(psA, 1.0 / N_OUT)

            # ---- passes B, C ----
            # a-reduce path runs self-contained per chunk on ONE engine
            # (cross-engine ping-pong per chunk costs ~1.5us stalls).
            # Pool-chunks: tmp/th folds on Pool, tiny reduce tail on DVE.

            for it in range(2):
                psS = pS.tile([B, J], f32, tag="psS", bufs=1)
                uh_live = {}

                def phase1(q):
                    uhq = [None] * 4
                    n_pool = 2 if q % 2 == 0 else 1
                    # aq collects the q's 4 aa slices; one batched blog add
                    aq = smalls.tile([128, 4 * N_OUT], f32, tag="aq")
                    pool_tails = []

                    def chunk_front(r):
                        psU = pU.tile([128, J], f32, tag="psU", bufs=3)
                        for jh in range(2):
                            nc.tensor.matmul(
                                psU[:, jh * 512:(jh + 1) * 512],
                                lhsT=uZP_t[32 * r:32 * r + 32,
                                           q * 2 * B:(q + 1) * 2 * B],
                                rhs=w_tiles[q][32 * r:32 * r + 32,
                                               jh * 512:(jh + 1) * 512],
                                start=True, stop=True,
                                tile_position=(32 * r, 0))
                        uh = uhp.tile([128, J], bf16, tag=f"uh{r}")
                        nc.scalar.mul(uh[:], psU[:], 1.0)
                        uhq[r] = uh
                        return uh

                    # Pool-chunks first: start Pool's long chains early
                    for r in range(n_pool):
                        uh = chunk_front(r)
                        tmp = tmpp.tile([128, J], bf16, tag="tmp")
                        nc.gpsimd.tensor_mul(tmp[:], uh[:], v_bf[:])
                        th = tmpp.tile([128, J // 2], bf16, tag="th")
                        nc.gpsimd.tensor_add(
                            th[:], tmp[:, 0:512], tmp[:, 512:1024])
                        th2 = tmpp.tile([128, J // 4], bf16, tag="th2")
                        nc.gpsimd.tensor_add(
                            th2[:], th[:, 0:256], th[:, 256:512])
                        th3 = tmpp.tile([128, J // 8], bf16, tag="th3")
                        nc.gpsimd.tensor_add(
                            th3[:], th2[:, 0:128], th2[:, 128:256])
                        pool_tails.append((r, th3))
                    # DVE-chunks: self-contained on DVE
                    for r in range(n_pool, 4):
                        uh = chunk_front(r)
                        tmp = tmpp.tile([128, J], bf16, tag="tmp")
                        nc.vector.tensor_mul(tmp[:], uh[:], v_bf[:])
                        th = tmpp.tile([128, J // 2], bf16, tag="thd")
                        nc.vector.tensor_add(
                            th[:], tmp[:, 0:512], tmp[:, 512:1024])
                        nc.vector.reduce_sum(
                            aq[:, r * N_OUT:(r + 1) * N_OUT],
                            th[:].rearrange("p (k o) -> p o k", o=N_OUT),
                            axis=AX.X)
                    # Pool-chunk DVE tails last (Pool chains done by now)
                    for r, th3 in pool_tails:
                        nc.vector.reduce_sum(
                            aq[:, r * N_OUT:(r + 1) * N_OUT],
                            th3[:].rearrange("p (k o) -> p o k", o=N_OUT),
                            axis=AX.X)
                    # single batched logits update for the whole q
                    bq = blog[:, q * 4 * N_OUT:(q + 1) * 4 * N_OUT]
                    nc.gpsimd.tensor_add(bq, bq, aq[:])
                    uh_live[q] = uhq

                eein_live = {}

                def phase2_head(q):
                    # softmax up to the exp INPUT (DVE): issued after the
                    # previous q's t2 work so the blog-add wait overlaps it
                    bq = blog[:, q * 4 * N_OUT:(q + 1) * 4 * N_OUT]
                    mx = smalls.tile([128, 4], f32, tag="mx")
                    nc.vector.reduce_max(
                        mx[:], bq.rearrange("p (r o) -> p r o", o=N_OUT),
                        axis=AX.X)
                    eein = smalls.tile([128, 4 * N_OUT], f32, tag="eein")
                    nc.vector.tensor_tensor(
                        eein[:].rearrange("p (r o) -> p r o", o=N_OUT),
                        bq.rearrange("p (r o) -> p r o", o=N_OUT),
                        mx[:].unsqueeze(2).broadcast_to([128, 4, N_OUT]),
                        op=ALU.subtract)
                    eein_live[q] = eein

                def phase2(q, last):
                    # exp (input long ready -> no ACT stall), tail, t2, merge
                    uhq = uh_live.pop(q)
                    eein = eein_live.pop(q)
                    ee = smalls.tile([128, 4 * N_OUT], f32, tag="ee")
                    nc.scalar.activation(ee[:], eein[:], AF.Exp)
                    sm = smalls.tile([128, 4], f32, tag="sm")
                    nc.vector.reduce_sum(
                        sm[:], ee[:].rearrange("p (r o) -> p r o", o=N_OUT),
                        axis=AX.X)
                    rc = smalls.tile([128, 4], f32, tag="rc")
                    nc.vector.reciprocal(rc[:], sm[:])
                    cc = smalls.tile([128, 4 * N_OUT], bf16, tag="cc")
                    nc.vector.tensor_tensor(
                        cc[:].rearrange("p (r o) -> p r o", o=N_OUT),
                        ee[:].rearrange("p (r o) -> p r o", o=N_OUT),
                        rc[:].unsqueeze(2).broadcast_to([128, 4, N_OUT]),
                        op=ALU.mult)
                    for r in range(4):
                        t2 = tmpp.tile([128, J], bf16, tag="t2")
                        nc.vector.tensor_tensor(
                            t2[:].rearrange("p (k o) -> p k o", o=N_OUT),
                            uhq[r][:].rearrange("p (k o) -> p k o", o=N_OUT),
                            cc[:, r * N_OUT:(r + 1) * N_OUT]
                            .unsqueeze(1).broadcast_to([128, D_OUT, N_OUT]),
                            op=ALU.mult)
                        for jh in range(2):
                            nc.tensor.matmul(
                                psS[:, jh * 512:(jh + 1) * 512],
                                lhsT=I2B_t[:],
                                rhs=t2[:, jh * 512:(jh + 1) * 512],
                                start=(q == 0 and r == 0),
                                stop=(last and r == 3))

                for q in range(QB):
                    phase1(q)
                    if q >= 1:
                        phase2(q - 1, last=False)
                    phase2_head(q)
                phase2(QB - 1, last=True)
                ar_squash(psS, 1.0)

            nc.sync.dma_start(v_d[:], v_t[:])

    nc.compile()
    return nc


def _get_program():
    if "nc" not in _CACHE:
        _CACHE["nc"] = _build_program()
    return _CACHE["nc"]


def kernel(u, W):
    from concourse.bass_utils import run_bass_kernel_spmd

    nc = _get_program()
    in_maps = _pack_inputs(np.asarray(u, np.float32), np.asarray(W, np.float32))
    res = run_bass_kernel_spmd(nc, in_maps, list(range(N_CORES)))
    v = res.results[0]["v_out"]
    # (k,o) layout -> [b, o, k]
    return np.ascontiguousarray(
        v.reshape(B, D_OUT, N_OUT).transpose(0, 2, 1))
